# revision 1
# baseline (speedup 1.0000x reference)
"""Bahdanau additive attention kernel for Trainium2 (8 NeuronCores, SPMD).

Problem: B=32, S=2048, ENC=DEC=ATT=1024 (fp32 inputs)
  u = enc @ U_a                [B,S,A]
  w = dec @ W_a                [B,1,A]
  e = tanh(w + u) @ v_t        [B,S,1]
  align = softmax(e, axis=1)
  context = align^T @ enc      [B,1,E]
  output = tanh([dec, context] @ ffn)   [B,1,D]
  returns (output, context)

Sharding: data-parallel over batch, 4 batches per core, weights replicated.

Per-core pipeline (streaming unit = q: 4 s-tiles = 512 seq positions):
  Pool SWDGE queue : cast loads only (weights fp32->bf16, enc fp32->bf16);
                     enc reads use consecutive-row descriptors (s relabeled)
  DVE              : enc bf16 -> fp8 cast; U8 = U*256 fp8 cast; reciprocal
  SP HWDGE         : enc fp8 xbar transposes (fp8 pairs viewed as u16)
  ACT HWDGE        : catT transposes + DRAM stores
  PE               : u-matmuls (fp8 DoubleRow), e-matmuls (fp8 DoubleRow
                     over tanh-pair tiles, v replicated 128-wide), expe
                     128-col transposes, ctx (bf16), ffn
  ACT              : tanh(u/256 + wT) fused fp8 out, exp (scale 1/32,
                     per-quarter accum)

The fp8 transpose packs two consecutive-e fp8 values in one u16 so the
xbar (16-bit only) moves half the bytes; U8 is loaded with the matching
(kb p c) row pairing so DoubleRow contracts the pairs directly.

Softmax path has no DMAs: exp writes the [1,S] expe row; 16 tiny PE
transposes ([1,128] -> [128,1] psum) build expe_cols for the ctx matmul.

Software pipelining: ctx(b) (PE transposes + ctx matmuls) is emitted after
u(b+1,q0) in PE program order so PE never head-of-line blocks on softmax;
batch 0's q0/q1 e-matmuls are deferred past u(0,1) so the PE queue never
waits on the W/v/dec weight loads.
"""

import numpy as np
import ml_dtypes

import concourse.bass as bass
import concourse.mybir as mybir
import concourse.tile as tile
from concourse import bacc
from concourse.bass_utils import run_bass_kernel_spmd

F32 = mybir.dt.float32
BF16 = mybir.dt.bfloat16
FP8 = mybir.dt.float8e4
AF = mybir.ActivationFunctionType
DR = mybir.MatmulPerfMode.DoubleRow

U_SCALE = 256.0
V_SCALE = 32.0

B, S, E, A, D = 32, 2048, 1024, 1024, 1024
NCORES = 8
NB = B // NCORES          # 4 batches per core
P = 128
KE = E // P               # 8 e-chunks (128 each)
KB = 4                    # e-pair blocks (256 e-values each) for DoubleRow
MA = A // P               # 8 output chunks over att dim
KD = D // P               # 8 contraction chunks over dec dim
ST = S // P               # 16 s-tiles per batch
NQ = 4                    # streaming units per batch
TQ = ST // NQ             # 4 s-tiles per unit (512 seq)
SQ = TQ * P               # 512 seq per unit
N512 = 512


def _build_kernel_body(tc, repeat=1):
    nc = tc.nc
    enc = nc.dram_tensor("enc", [NB, S, E], F32, kind="ExternalInput")
    dec = nc.dram_tensor("dec", [NB, D], F32, kind="ExternalInput")
    U_a = nc.dram_tensor("U_a", [E, A], F32, kind="ExternalInput")
    W_a = nc.dram_tensor("W_a", [D, A], F32, kind="ExternalInput")
    v_t = nc.dram_tensor("v_t", [A, 1], F32, kind="ExternalInput")
    ffn = nc.dram_tensor("ffn", [D + E, D], F32, kind="ExternalInput")
    out = nc.dram_tensor("out", [NB, D], F32, kind="ExternalOutput")
    ctx_out = nc.dram_tensor("ctx_out", [NB, E], F32, kind="ExternalOutput")
    for _ in range(repeat):
        _build_once(tc, enc, dec, U_a, W_a, v_t, ffn, out, ctx_out)


def _build_once(tc, enc, dec, U_a, W_a, v_t, ffn, out, ctx_out):
    nc = tc.nc
    # s relabeled so each partition reads 4 CONSECUTIVE dram rows (1 big
    # descriptor instead of 4): s = q*512 + p*4 + t. The relabeling flows
    # consistently through u/e/softmax/ctx (softmax is order-invariant and
    # every consumer uses the same tiling), so results are unchanged.
    enc_r = enc.rearrange("b (q p t) e -> b p q t e", q=NQ, p=P, t=TQ)
    # U rows paired (consecutive e) to match the fp8-in-u16 transpose:
    # U_sb[p, (kb c), a] = U[kb*256 + 2p + c, a]
    U_r = U_a.rearrange("(kb p c) a -> p kb c a", kb=KB, p=P, c=2)
    W_r = W_a.rearrange("(k p) a -> p k a", p=P)

    with (
        tc.tile_pool(name="weights", bufs=1) as weights,
        tc.tile_pool(name="big", bufs=1) as big,
        tc.tile_pool(name="enc_nat", bufs=4) as enc_nat_pool,
        tc.tile_pool(name="enc8", bufs=2) as enc8_pool,
        tc.tile_pool(name="encT8", bufs=3) as encT8_pool,
        tc.tile_pool(name="tanhp", bufs=6) as tanh_pool,
        tc.tile_pool(name="rows", bufs=1) as rows,
        tc.tile_pool(name="rows2", bufs=2) as rows2,
        tc.tile_pool(name="psum_u", bufs=3, space="PSUM") as psum_u,
        tc.tile_pool(name="psum_e", bufs=2, space="PSUM") as psum_e,
        tc.tile_pool(name="psum_c", bufs=1, space="PSUM") as psum_c,
        tc.tile_pool(name="psum_s", bufs=1, space="PSUM") as psum_s,
    ):
        # ---------------- Pool-queue cast loads (issue order matters) ----
        dec16 = rows.tile([16, D], BF16, tag="dec16")
        nc.vector.memset(dec16, 0.0)
        nc.gpsimd.dma_start(out=dec16[0:NB, :], in_=dec[:, :])

        def load_enc_pair(b, qp, split=False):
            """one DMA loads units (2*qp, 2*qp+1): [P, 2*TQ, E]. With
            split, two half-DMAs fill the same tile so the first unit's
            cast can start after half the transfer (startup pairs only)."""
            nat = enc_nat_pool.tile([P, 2 * TQ, E], BF16,
                                    name=f"nat_{b}_{qp}", tag="enc_nat")
            if split:
                for h in range(2):
                    nc.gpsimd.dma_start(
                        out=nat[:, h * TQ : (h + 1) * TQ, :],
                        in_=enc_r[b, :, 2 * qp + h, :, :],
                    )
            else:
                nc.gpsimd.dma_start(
                    out=nat, in_=enc_r[b, :, 2 * qp : 2 * qp + 2, :, :]
                )
            return nat

        v_sb = weights.tile([P, MA], BF16)
        nc.gpsimd.dma_start(out=v_sb, in_=v_t.rearrange("(m p) one -> p (m one)", p=P))
        nat00 = load_enc_pair(0, 0, split=True)
        # weight loads split into <=512-descriptor DMAs: a bigger one stalls
        # the SWDGE prep ring (1024 descriptors) and blocks the enc stream
        U_sb = weights.tile([P, KB, 2, A], BF16)
        nc.gpsimd.dma_start(out=U_sb[:, 0:2, :, :], in_=U_r[:, 0:2, :, :])
        nc.gpsimd.dma_start(out=U_sb[:, 2:4, :, :], in_=U_r[:, 2:4, :, :])
        W_sb = big.tile([P, KD, A], BF16, tag="big")
        nc.gpsimd.dma_start(out=W_sb[:, 0:4, :], in_=W_r[:, 0:4, :])
        nat01 = load_enc_pair(0, 1, split=True)
        nc.gpsimd.dma_start(out=W_sb[:, 4:8, :], in_=W_r[:, 4:8, :])
        nat_pre = {(0, 0): nat00, (0, 1): nat01}

        # ---------------- small shared tiles ----------------
        # catT[p, c, j] = cat[j, c*128+p] ; c 0..7 dec, 8..15 ctx.
        # The dec-half transpose sits right behind the dec load (both
        # prompt-completing), so it cannot poison the DMA issue window.
        catT = weights.tile([P, 2 * KE, 16], BF16)
        nc.sync.dma_start(out=catT[:, 0:KE, :], in_=dec16, transpose=True)
        ctx16 = rows.tile([16, E], BF16, tag="ctx16")
        nc.vector.memset(ctx16, 0.0)
        id1 = weights.tile([1, 1], F32)
        nc.vector.memset(id1, 1.0)

        # U8[p, kb, c, a] = U_sb[p, (kb c), a] * 256, fp8. The two halves
        # go to different engines so they don't serialize behind the enc
        # fp8 casts on the DVE queue at startup.
        U8 = weights.tile([P, KB, 2, A], FP8)
        for h, eng in ((0, nc.vector), (1, nc.gpsimd)):
            eng.tensor_scalar_mul(
                U8[:, 2 * h : 2 * h + 2, :, :].rearrange("p k c a -> p (k c a)"),
                U_sb[:, 2 * h : 2 * h + 2, :, :].rearrange("p k c a -> p (k c a)"),
                U_SCALE,
            )
        # v8[p, mm, j, cc] = v[(2mm+j)*128+p] * 32 in fp8 for all cc (DR
        # m-chunk pairs; *32 puts v's +-1/32 values in fp8's normal range,
        # exp's scale=1/32 compensates). The 128-wide column replication
        # mirrors U8's stationary shape — dual-fp8 Ldweights rejects
        # narrow stationaries — so the e-matmul output is 128 identical
        # rows of which exp reads row 0.
        v32 = weights.tile([P, MA], F32)
        nc.vector.tensor_scalar_mul(v32, v_sb, V_SCALE)
        zero128 = weights.tile([P, P], F32)
        nc.vector.memset(zero128, 0.0)
        v8 = weights.tile([P, MA // 2, 2, P], FP8)
        for mm in range(MA // 2):
            for j in range(2):
                nc.vector.tensor_scalar_add(
                    v8[:, mm, j, :], zero128, v32[:, 2 * mm + j : 2 * mm + j + 1]
                )

        # wT[p, m, b] = w[b, m*128+p] = sum_d W[d, m*128+p] dec[b, d]
        wT_ps = psum_c.tile([P, MA, NB], F32, tag="cvec")
        for m in range(MA):
            for k in range(KD):
                nc.tensor.matmul(
                    wT_ps[:, m, :],
                    lhsT=W_sb[:, k, m * P : (m + 1) * P],
                    rhs=catT[:, k, 0:NB],
                    start=(k == 0),
                    stop=(k == KD - 1),
                )
        wT = weights.tile([P, MA, NB], F32)
        nc.scalar.copy(wT, wT_ps)

        # ffn reuses W_sb's slot once W_a is consumed (loaded after batch 1
        # enc loads are queued; only needed at the very end)
        ffn_sb = None

        # ---------------- per-unit build helpers ----------------
        def build_u_block(b, q, rhs_all, e_ps, defer_e=False):
            """u matmuls (fp8 DR) + fused tanh (fp8 out) + DR e-matmuls.

            rhs_all is the unit's [p, kb, c, t, j] fp8 view of the pair
            transpose. tanh writes fp8 pairs (m even/odd in one tile) so
            the v-contraction also runs DoubleRow: e_ps accumulates
            32 * e (v8 = v*32); exp later rescales by 1/32.

            With defer_e the e-matmuls are returned as a thunk so the PE
            queue is not blocked on the w/v weight loads at startup."""
            th2s = []
            th2 = None
            for m in range(MA):
                u_ps = psum_u.tile([P, SQ], F32, name="u_ps", tag="u")
                for kb in range(KB):
                    nc.tensor.matmul(
                        u_ps,
                        lhsT=U8[:, kb, :, m * P : (m + 1) * P],
                        rhs=rhs_all[:, kb],
                        start=(kb == 0),
                        stop=(kb == KB - 1),
                        perf_mode=DR,
                    )
                if m % 2 == 0:
                    th2 = tanh_pool.tile([P, 2, SQ], FP8, name="th2", tag="th")
                nc.scalar.activation(
                    th2[:, m % 2, :], u_ps, AF.Tanh,
                    bias=wT[:, m, b : b + 1],
                    scale=1.0 / U_SCALE,
                )
                if m % 2 == 1:
                    th2s.append((m // 2, th2))
                    if not defer_e:
                        nc.tensor.matmul(
                            e_ps,
                            lhsT=v8[:, m // 2, :, :],
                            rhs=th2,
                            start=(m // 2 == 0),
                            stop=(m // 2 == MA // 2 - 1),
                            perf_mode=DR,
                        )
            if not defer_e:
                return None

            def emit_e(e_ps_late):
                for mm, t2 in th2s:
                    nc.tensor.matmul(
                        e_ps_late,
                        lhsT=v8[:, mm, :, :],
                        rhs=t2,
                        start=(mm == 0),
                        stop=(mm == MA // 2 - 1),
                        perf_mode=DR,
                    )

            return emit_e

        def build_exp_q(b, q, e_ps, expe, esum4):
            """exp + accum for one quarter (into the [1,S] expe row).
            e_ps holds 32*e (v8 scaling), undone by the exp scale."""
            nc.scalar.activation(
                expe[:, q * SQ : (q + 1) * SQ],
                e_ps[0:1, :],
                AF.Exp,
                scale=1.0 / V_SCALE,
                accum_out=esum4[:, q : q + 1],
            )

        def build_ctx(b, nats, esum4, expe):
            """expe PE-transposes + esum reduce + ctx matmuls + copy-out."""
            expeT_ps = psum_s.tile([P, ST], F32, name=f"expeT_{b}", tag="eT")
            for tg in range(ST):
                nc.tensor.transpose(
                    expeT_ps[:, tg : tg + 1],
                    expe[:, tg * P : (tg + 1) * P],
                    id1,
                )
            expe_cols = rows2.tile([P, ST], BF16, name=f"expec_{b}",
                                   tag="expe_cols")
            nc.vector.tensor_copy(expe_cols, expeT_ps)
            esum = rows2.tile([1, 1], F32, name=f"esumt_{b}", tag="esumt")
            nc.vector.tensor_reduce(esum, esum4, mybir.AxisListType.X,
                                    mybir.AluOpType.add)
            rsum = rows2.tile([1, 1], F32, name=f"rsum_{b}", tag="rsum")
            nc.vector.reciprocal(rsum, esum)
            ctx_ps = psum_c.tile([1, E], F32, name=f"ctx_ps_{b}", tag="cvec")
            for tg in range(ST):
                for n in range(2):
                    nc.tensor.matmul(
                        ctx_ps[:, n * N512 : (n + 1) * N512],
                        lhsT=expe_cols[:, tg : tg + 1],
                        rhs=nats[tg // (2 * TQ)][
                            :, tg % (2 * TQ), n * N512 : (n + 1) * N512
                        ],
                        start=(tg == 0),
                        stop=(tg == ST - 1),
                    )
            ctx_row = rows2.tile([1, E], F32, name=f"ctx_row_{b}", tag="ctx_row")
            nc.scalar.activation(ctx_row, ctx_ps, AF.Copy, scale=rsum)
            nc.scalar.dma_start(out=ctx_out[b : b + 1, :], in_=ctx_row)
            ctx_row16 = rows2.tile([1, E], BF16, name=f"ctx_row16_{b}",
                                   tag="ctx_row16")
            nc.scalar.copy(ctx_row16, ctx_row)
            nc.scalar.dma_start(out=ctx16[b : b + 1, :], in_=ctx_row16)

        # ---------------- main pipeline (software-pipelined) ----------
        # The DMA issue engine drains DMAs in global emission order, so a
        # transpose emitted right after its own unit's load serializes the
        # following loads behind its (cast-gated) completion. Stage A
        # (load + fp8 cast) therefore runs LAG units ahead of stage B
        # (transpose + u-block): every DMA's dependency is long resolved
        # when its issue turn comes.
        LAG = 2
        units = [(b, q) for b in range(NB) for q in range(NQ)]
        e8_u = {}
        nat_u = {}
        bst = {}

        def batch_state(b):
            if b not in bst:
                bst[b] = {
                    "nats": [],
                    "expe": rows2.tile([1, S], F32, name=f"expe_{b}",
                                       tag="expe"),
                    "esum4": rows2.tile([1, NQ], F32, name=f"esum4_{b}",
                                        tag="esum4"),
                    "e_ps": {},
                    "deferred": [],
                }
            return bst[b]

        def get_e_ps(b, qq):
            eps = batch_state(b)["e_ps"]
            if qq not in eps:
                eps[qq] = psum_e.tile([P, N512], F32,
                                      name=f"e_ps_{b}_{qq}", tag="e")
            return eps[qq]

        def flush_deferred(b):
            st = batch_state(b)
            for dq, thunk in st["deferred"]:
                dps = get_e_ps(b, dq)
                thunk(dps)
                build_exp_q(b, dq, dps, st["expe"], st["esum4"])
            st["deferred"] = []

        pending = None  # (b, nats, esum4, expe) awaiting ctx emission
        for i in range(len(units) + LAG):
            if i < len(units):
                # ---- stage A: pair load + cast (pair-shared fp8 tile) ----
                b, q = units[i]
                if i % 2 == 0:
                    nat = nat_pre.get((b, q // 2))
                    if nat is None:
                        nat = load_enc_pair(b, q // 2)
                    batch_state(b)["nats"].append(nat)
                    nat_u[i // 2] = nat
                    e8p = enc8_pool.tile([P, 2 * TQ * E], FP8,
                                         name=f"e8_{b}_{q}", tag="e8")
                    e8_u[i // 2] = e8p
                nc.vector.tensor_copy(
                    e8_u[i // 2][:, (i % 2) * TQ * E : (i % 2 + 1) * TQ * E],
                    nat_u[i // 2][:, (i % 2) * TQ : (i % 2 + 1) * TQ, :]
                    .rearrange("p t e -> p (t e)"),
                )
                if i == len(units) - 1:
                    ffn_sb = big.tile([P, 2 * KE, D], BF16, tag="big")
                    ffn_r = ffn.rearrange("(k p) d -> p k d", p=P)
                    for c in range(4):
                        nc.gpsimd.dma_start(
                            out=ffn_sb[:, c * 4 : (c + 1) * 4, :],
                            in_=ffn_r[:, c * 4 : (c + 1) * 4, :],
                        )
            j = i - LAG
            if j < 0 or j % 2 == 0:
                continue
            # ---- stage B: one pair transpose + both units' u-blocks ----
            e8p = e8_u.pop(j // 2)
            eTp = encT8_pool.tile([P, 8 * TQ, P], mybir.dt.uint16,
                                  name=f"eT_{j // 2}", tag="encT8")
            nc.sync.dma_start(
                out=eTp, in_=e8p.bitcast(mybir.dt.uint16), transpose=True
            )
            rhs_pair = eTp[:, :, :].bitcast(FP8).rearrange(
                "p (qq t k) (j c) -> p qq k c t j", qq=2, t=TQ, k=KB, c=2
            )
            for jj in (j - 1, j):
                b, q = units[jj]
                st = batch_state(b)
                if q == 2:
                    # flush batch-0 deferred work before psum slots recycle
                    flush_deferred(b)
                rhs_all = rhs_pair[:, jj - (j - 1)]
                # batch 0 q0/q1: keep the PE queue free of e-matmuls until
                # the W/dec loads and the w-matmuls have certainly landed
                defer = b == 0 and q < 2
                if not defer:
                    e_ps = get_e_ps(b, q)
                    build_u_block(b, q, rhs_all, e_ps, defer_e=False)
                    build_exp_q(b, q, e_ps, st["expe"], st["esum4"])
                else:
                    thunk = build_u_block(b, q, rhs_all, None, defer_e=True)
                    st["deferred"].append((q, thunk))
                if pending is not None and q == 0:
                    # ctx of the previous batch lands behind u(b, q0) on PE
                    build_ctx(*pending)
                    pending = None
                if q == NQ - 1:
                    flush_deferred(b)
                    pending = (b, st["nats"], st["esum4"], st["expe"])
        build_ctx(*pending)

        # ---------------- final ffn (all batches at once) ----------------
        nc.scalar.dma_start(out=catT[:, KE : 2 * KE, :], in_=ctx16,
                            transpose=True)
        out_ps = psum_c.tile([NB, D], F32, tag="cvec")
        for c in range(2 * KE):
            for n in range(2):
                nc.tensor.matmul(
                    out_ps[:, n * N512 : (n + 1) * N512],
                    lhsT=catT[:, c, 0:NB],
                    rhs=ffn_sb[:, c, n * N512 : (n + 1) * N512],
                    start=(c == 0),
                    stop=(c == 2 * KE - 1),
                )
        out_sb = weights.tile([NB, D], F32)
        nc.scalar.activation(out_sb, out_ps, AF.Tanh)
        nc.scalar.dma_start(out=out[:, :], in_=out_sb)


_NC_CACHE = None


def _get_nc(repeat=1):
    global _NC_CACHE
    if repeat != 1:
        nc = bacc.Bacc(None, target_bir_lowering=False)
        with tile.TileContext(nc) as tc:
            _build_kernel_body(tc, repeat=repeat)
        nc.compile()
        return nc
    if _NC_CACHE is None:
        nc = bacc.Bacc(None, target_bir_lowering=False)
        with tile.TileContext(nc) as tc:
            _build_kernel_body(tc)
        nc.compile()
        _NC_CACHE = nc
    return _NC_CACHE


def kernel(encoder_hidden_states, decoder_hidden_state, U_a, W_a, v_t, ffn,
           _trace=False):
    enc = np.ascontiguousarray(np.asarray(encoder_hidden_states, dtype=np.float32))
    dec = np.ascontiguousarray(
        np.asarray(decoder_hidden_state, dtype=np.float32).reshape(B, D)
    )
    U = np.ascontiguousarray(np.asarray(U_a, dtype=np.float32))
    W = np.ascontiguousarray(np.asarray(W_a, dtype=np.float32))
    v = np.ascontiguousarray(np.asarray(v_t, dtype=np.float32))
    F = np.ascontiguousarray(np.asarray(ffn, dtype=np.float32))

    nc = _get_nc()
    in_maps = []
    for c in range(NCORES):
        sl = slice(c * NB, (c + 1) * NB)
        in_maps.append(
            {
                "enc": enc[sl],
                "dec": dec[sl],
                "U_a": U,
                "W_a": W,
                "v_t": v,
                "ffn": F,
            }
        )
    res = run_bass_kernel_spmd(nc, in_maps, core_ids=list(range(NCORES)),
                               trace=_trace)

    output = np.empty((B, 1, D), dtype=np.float32)
    context = np.empty((B, 1, E), dtype=np.float32)
    for c in range(NCORES):
        sl = slice(c * NB, (c + 1) * NB)
        output[sl, 0, :] = res.results[c]["out"]
        context[sl, 0, :] = res.results[c]["ctx_out"]
    if _trace:
        return (output, context), res
    return (output, context)


if __name__ == "__main__":
    import reference

    inputs = {k: np.asarray(v) for k, v in reference.setup_inputs().items()}
    (o, c) = kernel(**inputs)
    print("output", o.shape, o.dtype, "context", c.shape, c.dtype)



# revision 92
# speedup vs baseline: 1.2923x; 1.2923x over previous
"""Bahdanau additive attention kernel for Trainium2 (8 NeuronCores, SPMD).

Problem: B=32, S=2048, ENC=DEC=ATT=1024 (fp32 inputs)
  u = enc @ U_a                [B,S,A]
  w = dec @ W_a                [B,1,A]
  e = tanh(w + u) @ v_t        [B,S,1]
  align = softmax(e, axis=1)
  context = align^T @ enc      [B,1,E]
  output = tanh([dec, context] @ ffn)   [B,1,D]
  returns (output, context)

Sharding: data-parallel over batch, 4 batches per core, weights replicated.

v2 design (vs v1): enc is DMA-cast fp32->fp8 directly (no bf16 copy, no
DVE cast); the ctx matmul runs fp8 DoubleRow against the same fp8 enc
tiles with softmax weights replicated 128-wide (tiny PE replicate
matmuls + DVE fp8 copies); all non-activation work is kept off the ACT
engine so it streams pure tanh+exp; e-matmuls/exp of unit k are emitted
after unit k+1's u-block so PE never waits on the tanh lag.

Per-core engine budget (TimelineSim): ACT ~93us (128 tanh + 16 exp),
PE ~80us (512 u-MM fp8 DR + e/ctx/ffn/w), DMA device ~77us (enc fp8
loads 23 + xbar transposes 29 + U/W/ffn 23).
"""

import numpy as np
import ml_dtypes

import concourse.bass as bass
import concourse.mybir as mybir
import concourse.tile as tile
from concourse import bacc
from concourse.bass_utils import run_bass_kernel_spmd

F32 = mybir.dt.float32
BF16 = mybir.dt.bfloat16
FP8 = mybir.dt.float8e4
U16 = mybir.dt.uint16
AF = mybir.ActivationFunctionType
DR = mybir.MatmulPerfMode.DoubleRow

U_SCALE = 1.0   # U loaded as raw fp8 (no scale)
V_SCALE = 32.0

B, S, E, A, D = 32, 2048, 1024, 1024, 1024
NCORES = 8
NB = B // NCORES          # 4 batches per core
P = 128
KE = E // P               # 8 e-chunks (128 each)
KB = 4                    # e-pair blocks (256 e-values each) for DoubleRow
MA = A // P               # 8 output chunks over att dim
KD = D // P               # 8 contraction chunks over dec dim
ST = S // P               # 16 s-tiles per batch
NQ = 4                    # units per batch
TQ = ST // NQ             # 4 s-tiles per unit (512 seq)
SQ = TQ * P               # 512 seq per unit
N512 = 512
NU = NB * NQ              # 16 units per core
NP = NU // 2              # 8 pair (2-unit) load/transpose groups


def _build_kernel_body(tc, repeat=1):
    nc = tc.nc
    enc = nc.dram_tensor("enc", [NB, S, E], F32, kind="ExternalInput")
    dec = nc.dram_tensor("dec", [NB, D], F32, kind="ExternalInput")
    U_a = nc.dram_tensor("U_a", [E, A], F32, kind="ExternalInput")
    W_a = nc.dram_tensor("W_a", [D, A], F32, kind="ExternalInput")
    v_t = nc.dram_tensor("v_t", [A, 1], F32, kind="ExternalInput")
    ffn = nc.dram_tensor("ffn", [D + E, D], F32, kind="ExternalInput")
    out = nc.dram_tensor("out", [NB, D], F32, kind="ExternalOutput")
    ctx_out = nc.dram_tensor("ctx_out", [NB, E], F32, kind="ExternalOutput")
    for _ in range(repeat):
        _build_once(tc, enc, dec, U_a, W_a, v_t, ffn, out, ctx_out)


def _build_once(tc, enc, dec, U_a, W_a, v_t, ffn, out, ctx_out):
    nc = tc.nc
    # s relabeled so each partition reads 4 CONSECUTIVE dram rows (one big
    # descriptor instead of 4): s = q*512 + p*4 + t. The relabeling flows
    # consistently through u/e/softmax/ctx (softmax is order-invariant and
    # every consumer uses the same tiling), so results are unchanged.
    enc_r = enc.rearrange("b (q p t) e -> b p q t e", q=NQ, p=P, t=TQ)
    # U rows paired (consecutive e) to match the fp8-in-u16 transpose:
    # U_sb[p, (kb c), a] = U[kb*256 + 2p + c, a]
    U_r = U_a.rearrange("(kb p c) a -> p kb c a", kb=KB, p=P, c=2)
    # W/ffn contractions relabeled k-major (d = p*8 + k) so each partition
    # reads consecutive dram rows -> 128-descriptor DMAs that don't choke
    # the SWDGE prep ring. The dec/ctx transposes below use matching
    # strided views, so results are unchanged.
    W_r = W_a.rearrange("(p k) a -> p k a", p=P)
    ffn_r = ffn.rearrange("(hf p c) d -> p hf c d", hf=2, p=P)

    with (
        tc.tile_pool(name="weights", bufs=1) as weights,
        tc.tile_pool(name="enc8", bufs=6) as enc8_pool,
        tc.tile_pool(name="encT8", bufs=3) as encT8_pool,
        tc.tile_pool(name="tanhp", bufs=9) as tanh_pool,
        tc.tile_pool(name="rows", bufs=1) as rows,
        tc.tile_pool(name="rows2", bufs=2) as rows2,
        tc.tile_pool(name="rows4", bufs=4) as rows4,
        tc.tile_pool(name="qtiles", bufs=1) as qtiles,
        tc.tile_pool(name="psum_u", bufs=3, space="PSUM") as psum_u,
        tc.tile_pool(name="psum_e", bufs=3, space="PSUM") as psum_e,
        tc.tile_pool(name="psum_c", bufs=1, space="PSUM") as psum_c,
    ):
        # ---------------- Pool-queue loads (device order matters) --------
        # Startup chain to the first tanh: enc(0 unit 0) -> T(0a) while
        # U(a-half0)+cast and W(a-half0)+wT land -> first u/tanh ~9us.
        # Weight a-halves let m0-3 start before the second halves land.
        # Pair 0 uses separate per-unit tiles: dependency tracking is
        # tile-granular, so a shared tile would make T(0a) wait both
        # half-loads.
        # dec rides the SP HWDGE queue as fp32 (no Pool prep, no cast) and
        # is transposed on the idle PE instead of the xbar; one flat [1, B*D]
        # tile (single descriptor, single DMA-window slot) keeps every
        # PE-transpose input at partition 0.
        dec_flat = rows.tile([1, NB * D], F32, tag="dec32")
        nc.sync.dma_start(out=dec_flat, in_=dec[:, :])

        # U loads straight to fp8 (gpsimd DMA cast, no scale): u already
        # tolerates ~3% fp8 noise on enc; the raw-range U quantization adds
        # ~1.2x to that one term and saves the bf16 load + DVE cast from
        # the startup critical path. Per-kb tiles keep deps precise.
        U8k = [weights.tile([P, 2, A], FP8, name=f"U8_{kb}")
               for kb in range(KB)]
        W_sb = weights.tile([P, KD, A], BF16)
        v_sb = weights.tile([P, MA], BF16)

        def load_U(kb):
            # full-a per-kb: partition p reads 2 consecutive dram rows (8KB)
            # per descriptor -> 128 descriptors/DMA (SWDGE-ring friendly)
            nc.gpsimd.dma_start(out=U8k[kb], in_=U_r[:, kb])

        def load_W():
            for k in (0, 2, 4, 6):
                nc.gpsimd.dma_start(
                    out=W_sb[:, k : k + 2, :], in_=W_r[:, k : k + 2, :]
                )

        # unit_nat[k]: t -> (tile, local_t) natural fp8 enc for unit k
        unit_nat = {}

        def load_enc_pair(pp):
            nat = enc8_pool.tile([P, 2 * TQ, E], FP8, name=f"nat_{pp}",
                                 tag="enc8")
            b, q = divmod(2 * pp, NQ)
            nc.gpsimd.dma_start(out=nat, in_=enc_r[b, :, q : q + 2, :, :])
            unit_nat[2 * pp] = lambda t, nat=nat: (nat, t)
            unit_nat[2 * pp + 1] = lambda t, nat=nat: (nat, TQ + t)

        def load_enc_unit(k):
            nat = qtiles.tile([P, TQ, E], FP8, name=f"natu_{k}",
                              tag=f"enc8u{k}")
            b, q = divmod(k, NQ)
            nc.gpsimd.dma_start(out=nat, in_=enc_r[b, :, q, :, :])
            unit_nat[k] = lambda t, nat=nat: (nat, t)

        def load_enc_half_unit(k, h):
            # [P, 2, E] quarter tiles: unit 0 splits so its first transpose
            # (which the whole startup DMA window drains behind) completes
            # ~5us earlier
            nat = qtiles.tile([P, 2, E], FP8, name=f"natq_{k}_{h}",
                              tag=f"enc8q{h}")
            b, q = divmod(k, NQ)
            nc.gpsimd.dma_start(
                out=nat, in_=enc_r[b, :, q, 2 * h : 2 * h + 2, :]
            )
            return nat

        load_enc_unit(0)
        for kb in range(KB):
            load_U(kb)
        load_W()
        load_enc_unit(1)
        nc.gpsimd.dma_start(
            out=v_sb, in_=v_t.rearrange("(m p) one -> p (m one)", p=P)
        )
        load_enc_pair(1)
        # remaining enc pairs + ffn are emitted inside the main loop.
        # NOTE on DMA ordering: the scheduler issues DMAs through a bounded
        # in-flight window in program order, so every DMA/transpose must
        # have its dependencies long-resolved by the time its turn comes;
        # transposes are emitted one pair-slot behind their input load, and
        # all pair tiles are fresh buffers (no WAR waits in the stream).

        # ---------------- small shared tiles ----------------
        # catT[p, c, j] = cat[j, c*128+p] ; c 0..7 dec, 8..15 ctx (bf16,
        # written per-batch from PE transposes of ctx_row).
        catT = weights.tile([P, 2 * KE, NB], BF16)
        ones128 = weights.tile([1, P], BF16)
        nc.vector.memset(ones128, 1.0)
        id1 = weights.tile([1, 1], F32)
        nc.vector.memset(id1, 1.0)
        # dummy activation so the 1.3us LoadActFuncSet runs at t~0 instead
        # of right before the first real tanh
        act_warm = weights.tile([1, 1], F32)
        nc.scalar.activation(act_warm, id1, AF.Tanh)
        # dec transpose on PE, k-major to match W_r's row labeling:
        # catT[p, k, j] = dec[j, p*8+k]; per-row [1,128]->[128,1] transposes
        decT_ps = psum_u.tile([P, KE, NB], F32, tag="u")
        dec_kv = dec_flat.rearrange("o (j p2 k) -> o j k p2", j=NB, k=KD)
        for j in range(NB):
            for k in range(KD):
                nc.tensor.transpose(
                    decT_ps[:, k, j : j + 1], dec_kv[:, j, k, :], id1
                )
        nc.vector.tensor_copy(
            catT[:, 0:KE, :].rearrange("p c j -> p (c j)"),
            decT_ps.rearrange("p c j -> p (c j)"),
        )

        # v8[p, mm, j, cc] = v[(2mm+j)*128+p] * 32 fp8, replicated 128 wide
        # (dual-fp8 Ldweights rejects narrow stationaries); the e-matmul
        # output is 128 identical rows of which exp reads row 0.
        v32 = weights.tile([P, MA], F32)
        nc.vector.tensor_scalar_mul(v32, v_sb, V_SCALE)
        zero128 = weights.tile([P, P], F32)
        nc.vector.memset(zero128, 0.0)
        v8 = weights.tile([P, MA // 2, 2, P], FP8)
        for mm in range(MA // 2):
            for j in range(2):
                nc.vector.tensor_scalar_add(
                    v8[:, mm, j, :], zero128, v32[:, 2 * mm + j : 2 * mm + j + 1]
                )

        # wT[p, m, b] = w[b, m*128+p], emitted per a-quarter (m-pair) so
        # tanh(m0-1) isn't gated on the whole W stream; later quarters are
        # emitted from inside the main loop (after u-blocks) so a pending
        # W DMA never head-blocks the in-order PE queue.
        wT_ps = psum_c.tile([P, MA, NB], F32, tag="cvec")
        wT = weights.tile([P, MA, NB], F32)

        for m in range(MA):
            for k in range(KD):
                nc.tensor.matmul(
                    wT_ps[:, m, :],
                    lhsT=W_sb[:, k, m * P : (m + 1) * P],
                    rhs=catT[:, k, :],
                    start=(k == 0),
                    stop=(k == KD - 1),
                )
        nc.vector.tensor_copy(
            wT.rearrange("p m b -> p (m b)"),
            wT_ps.rearrange("p m b -> p (m b)"),
        )

        # ffn_sb[p, hf, c, d] = ffn[hf*1024 + p*8 + c, d] (k-major halves:
        # hf=0 dec rows, hf=1 ctx rows); 4 DMAs of 64 big descriptors
        ffn_sb = weights.tile([P, 2, KD, D], BF16)

        def load_ffn():
            for hf in range(2):
                for c in (0, 4):
                    nc.gpsimd.dma_start(
                        out=ffn_sb[:, hf, c : c + 4, :],
                        in_=ffn_r[:, hf, c : c + 4, :],
                    )

        # ---------------- transposes (SP HWDGE queue) ----------------
        # eTp[p, (qq t k), (j c)]: fp8 pairs viewed as u16 through the xbar.
        # rhs_view[k] is a list of (view, t0, tn) segments; view dims are
        # [p, kb, c, t, j] fp8 slices of the transposed result.
        rhs_view = {}

        def emit_T_pair(pp):
            eTp = encT8_pool.tile([P, 8 * TQ, P], U16, name=f"eT_{pp}",
                                  tag="encT8")
            src = unit_nat[2 * pp](0)[0].rearrange(
                "p t e -> p (t e)").bitcast(U16)
            nc.sync.dma_start(out=eTp, in_=src, transpose=True)
            pair_view = eTp[:, :, :].bitcast(FP8).rearrange(
                "p (qq t k) (j c) -> p qq k c t j", qq=2, t=TQ, k=KB, c=2
            )
            rhs_view[2 * pp] = [(pair_view[:, 0], 0, TQ)]
            rhs_view[2 * pp + 1] = [(pair_view[:, 1], 0, TQ)]

        def emit_T_unit(k):
            eTu = qtiles.tile([P, 4 * TQ, P], U16, name=f"eTu_{k}",
                              tag=f"encT8u{k}")
            src = unit_nat[k](0)[0].rearrange("p t e -> p (t e)").bitcast(U16)
            nc.sync.dma_start(out=eTu, in_=src, transpose=True)
            rhs_view[k] = [(eTu[:, :, :].bitcast(FP8).rearrange(
                "p (t k) (j c) -> p k c t j", t=TQ, k=KB, c=2
            ), 0, TQ)]

        def emit_T_half_unit(k, h, nat):
            eTq = qtiles.tile([P, 2 * TQ, P], U16, name=f"eTq_{k}_{h}",
                              tag=f"encT8q{h}")
            src = nat.rearrange("p t e -> p (t e)").bitcast(U16)
            nc.sync.dma_start(out=eTq, in_=src, transpose=True)
            view = eTq[:, :, :].bitcast(FP8).rearrange(
                "p (t k) (j c) -> p k c t j", t=2, k=KB, c=2
            )
            rhs_view.setdefault(k, []).append((view, 2 * h, 2))

        emit_T_unit(0)
        emit_T_unit(1)

        # ---------------- per-unit / per-batch helpers ----------------
        bst = {}

        def batch_state(b):
            if b not in bst:
                bst[b] = {
                    "expe": rows2.tile([1, S], BF16, name=f"expe_{b}",
                                       tag="expe"),
                    "esum4": rows2.tile([1, NQ], F32, name=f"esum4_{b}",
                                        tag="esum4"),
                    "th2s": {},
                    "e_ps": {},
                }
            return bst[b]

        def build_u_block(k):
            """u matmuls (fp8 DR) + fused tanh (fp8 out) for unit k."""
            b, q = divmod(k, NQ)
            st = batch_state(b)
            th2s = []
            th2 = None
            for m in range(MA):
                u_ps = psum_u.tile([P, SQ], F32, name="u_ps", tag="u")
                for (seg, t0, tn) in rhs_view[k]:
                    for kb in range(KB):
                        nc.tensor.matmul(
                            u_ps[:, t0 * P : (t0 + tn) * P],
                            lhsT=U8k[kb][:, :, m * P : (m + 1) * P],
                            rhs=seg[:, kb],
                            start=(kb == 0),
                            stop=(kb == KB - 1),
                            perf_mode=DR,
                        )
                if m % 2 == 0:
                    th2 = tanh_pool.tile([P, 2, SQ], FP8, name="th2", tag="th")
                nc.scalar.activation(
                    th2[:, m % 2, :], u_ps, AF.Tanh,
                    bias=wT[:, m, b : b + 1],
                    scale=1.0 / U_SCALE,
                )
                if m % 2 == 1:
                    th2s.append(th2)
            st["th2s"][q] = th2s

        def emit_e_exp(k):
            """e-matmuls (fp8 DR over tanh pairs) + exp for unit k; emitted
            one unit late so PE/ACT never wait on the tanh lag."""
            b, q = divmod(k, NQ)
            st = batch_state(b)
            e_ps = psum_e.tile([P, SQ], F32, name=f"e_ps_{k}", tag="eps")
            for mm, t2 in enumerate(st["th2s"].pop(q)):
                nc.tensor.matmul(
                    e_ps,
                    lhsT=v8[:, mm, :, :],
                    rhs=t2,
                    start=(mm == 0),
                    stop=(mm == MA // 2 - 1),
                    perf_mode=DR,
                )
            # e_ps holds 32*e (v8 scaling), undone by the exp scale
            nc.scalar.activation(
                st["expe"][:, q * SQ : (q + 1) * SQ],
                e_ps[0:1, :],
                AF.Exp,
                scale=1.0 / V_SCALE,
                accum_out=st["esum4"][:, q : q + 1],
            )

        def emit_rep(b):
            """PE replicate-matmuls: expe8_rep[p, tg, :] = expe[tg*128+p]
            (fp8, 128-wide) in 4 single-bank chunks, DVE-copied to sbuf.
            rep8b holds the fp8 cast RESIDUAL (expe - fp8(expe)): the ctx
            matmul accumulates both, wiping the softmax-weight quantization
            error (~30% of ctx's error budget) for one extra DVE op and 16
            extra 107ns DR matmuls per batch."""
            alloc_rep(b)
            emit_rep_chunks(b, range(4))
            emit_sums(b)

        def alloc_rep(b):
            st = batch_state(b)
            st["rep8"] = rows2.tile([P, ST, P], FP8, name=f"rep8_{b}",
                                    tag="rep8")
            st["rep8b"] = rows2.tile([P, ST, P], FP8, name=f"rep8b_{b}",
                                     tag="rep8b")

        def emit_rep_chunks(b, chunks):
            st = batch_state(b)
            rep8, rep8b = st["rep8"], st["rep8b"]
            for c in chunks:
                rep_ps = psum_e.tile([P, 4 * P], F32, name=f"rep_ps_{b}_{c}",
                                     tag="eps")
                for t in range(4):
                    tg = c * 4 + t
                    nc.tensor.matmul(
                        rep_ps[:, t * P : (t + 1) * P],
                        lhsT=st["expe"][:, tg * P : (tg + 1) * P],
                        rhs=ones128,
                        start=True,
                        stop=True,
                    )
                sl = rep8[:, 4 * c : 4 * (c + 1), :].rearrange(
                    "p t j -> p (t j)")
                nc.vector.tensor_copy(sl, rep_ps)
                nc.vector.tensor_tensor(
                    rep8b[:, 4 * c : 4 * (c + 1), :].rearrange(
                        "p t j -> p (t j)"),
                    rep_ps, sl, mybir.AluOpType.subtract,
                )

        def emit_sums(b):
            st = batch_state(b)
            esum = rows2.tile([1, 1], F32, name=f"esumt_{b}", tag="esumt")
            nc.vector.tensor_reduce(esum, st["esum4"], mybir.AxisListType.X,
                                    mybir.AluOpType.add)
            rsum = rows2.tile([1, 1], F32, name=f"rsum_{b}", tag="rsum")
            nc.vector.reciprocal(rsum, esum)
            st["rsum"] = rsum

        def emit_ctx(b):
            """fp8 DoubleRow ctx matmuls + scale/copy-out (DVE)."""
            emit_ctx_mms(b, range(ST // 2))
            emit_ctx_fin(b)

        def emit_ctx_mms(b, urange):
            st = batch_state(b)
            if "ctx_ps" not in st:
                st["ctx_ps"] = psum_c.tile([P, 2, N512], F32,
                                           name=f"ctx_ps_{b}", tag="cvec")
            ctx_ps = st["ctx_ps"]
            for n in range(2):
                for h, rep in enumerate((st["rep8"], st["rep8b"])):
                    for u in urange:
                        q, t = divmod(2 * u, NQ)
                        nat, lt = unit_nat[b * NQ + q](t)
                        nc.tensor.matmul(
                            ctx_ps[:, n, :],
                            lhsT=rep[:, 2 * u : 2 * u + 2, :],
                            rhs=nat[:, lt : lt + 2,
                                    n * N512 : (n + 1) * N512],
                            start=(h == 0 and u == 0),
                            stop=(h == 1 and u == ST // 2 - 1),
                            perf_mode=DR,
                            skip_group_check=True,
                        )

        def emit_ctx_fin(b):
            st = batch_state(b)
            ctx_ps = st["ctx_ps"]
            ctx_row = rows4.tile([1, E], F32, name=f"ctx_row_{b}",
                                 tag="ctx_row")
            st["ctx_row"] = ctx_row
            nc.vector.tensor_scalar_mul(
                ctx_row, ctx_ps[0:1, :, :].rearrange("o n f -> o (n f)"),
                st["rsum"],
            )
            # ctx column of catT via PE transposes (keeps the tail off the
            # xbar and the SP queue)
            # k-major strided views to match ffn_r's ctx-half row labeling
            ctx_kv = ctx_row.rearrange("o (p2 c) -> o c p2", c=KE)
            ctxT_ps = psum_e.tile([P, KE], F32, name=f"ctxT_{b}", tag="eps")
            for c in range(KE):
                nc.tensor.transpose(
                    ctxT_ps[:, c : c + 1], ctx_kv[:, c, :], id1
                )
            nc.vector.tensor_copy(
                catT[:, KE : 2 * KE, b : b + 1].rearrange("p c j -> p (c j)"),
                ctxT_ps,
            )

        # ---------------- main pipeline ----------------
        # Unit k's u-block is emitted at slot k; e/exp of k-1 after it;
        # rep(b-1) after slot 4b+1's u-block; ctx(b-1) after slot 4b+2's
        # (so the DVE rep8 copies overlap u(4b+2)'s execution).
        for k in range(NU):
            if k % 2 == 0 and k // 2 + 2 < NP:
                load_enc_pair(k // 2 + 2)
            if k % 2 == 0 and 1 <= k // 2 + 1 < NP:
                emit_T_pair(k // 2 + 1)
            if k == 12:
                load_ffn()
            build_u_block(k)
            if k >= 1:
                emit_e_exp(k - 1)
            if k >= 5 and k % NQ == 1:
                emit_rep(k // NQ - 1)
            if k >= 6 and k % NQ == 2:
                emit_ctx(k // NQ - 1)
            # last batch: front-run softmax replication + the q0/q1 half of
            # the ctx matmuls so only ~half the chain trails the last tanh
            if k == NU - 2:
                alloc_rep(NB - 1)
                emit_rep_chunks(NB - 1, (0, 1))
            if k == NU - 1:
                emit_rep_chunks(NB - 1, (2,))
                emit_ctx_mms(NB - 1, (0, 1, 2, 3))
        emit_e_exp(NU - 1)
        emit_rep_chunks(NB - 1, (3,))
        emit_sums(NB - 1)
        emit_ctx_mms(NB - 1, (4, 5, 6, 7))
        emit_ctx_fin(NB - 1)

        # ---------------- final ffn (all batches at once) ----------------
        out_ps = psum_c.tile([NB, D], F32, tag="cvec")
        for hf in range(2):
            for c in range(KD):
                for n in range(2):
                    nc.tensor.matmul(
                        out_ps[:, n * N512 : (n + 1) * N512],
                        lhsT=catT[:, hf * KE + c, :],
                        rhs=ffn_sb[:, hf, c, n * N512 : (n + 1) * N512],
                        start=(hf == 0 and c == 0),
                        stop=(hf == 1 and c == KD - 1),
                    )
        out_sb = weights.tile([NB, D], F32)
        nc.scalar.activation(out_sb, out_ps, AF.Tanh)
        nc.scalar.dma_start(out=out[:, :], in_=out_sb)
        # ctx stores last: tiny DMAs whose deps resolved long ago, so they
        # never head-block the issue window mid-stream
        for b in range(NB):
            nc.scalar.dma_start(out=ctx_out[b : b + 1, :],
                                in_=bst[b]["ctx_row"])


_NC_CACHE = None


def _get_nc(repeat=1):
    global _NC_CACHE
    if repeat != 1:
        nc = bacc.Bacc(None, target_bir_lowering=False)
        with tile.TileContext(nc) as tc:
            _build_kernel_body(tc, repeat=repeat)
        nc.compile()
        return nc
    if _NC_CACHE is None:
        nc = bacc.Bacc(None, target_bir_lowering=False)
        with tile.TileContext(nc) as tc:
            _build_kernel_body(tc)
        nc.compile()
        _NC_CACHE = nc
    return _NC_CACHE


def kernel(encoder_hidden_states, decoder_hidden_state, U_a, W_a, v_t, ffn,
           _trace=False):
    enc = np.ascontiguousarray(np.asarray(encoder_hidden_states, dtype=np.float32))
    dec = np.ascontiguousarray(
        np.asarray(decoder_hidden_state, dtype=np.float32).reshape(B, D)
    )
    U = np.ascontiguousarray(np.asarray(U_a, dtype=np.float32))
    W = np.ascontiguousarray(np.asarray(W_a, dtype=np.float32))
    v = np.ascontiguousarray(np.asarray(v_t, dtype=np.float32))
    F = np.ascontiguousarray(np.asarray(ffn, dtype=np.float32))

    nc = _get_nc()
    in_maps = []
    for c in range(NCORES):
        sl = slice(c * NB, (c + 1) * NB)
        in_maps.append(
            {
                "enc": enc[sl],
                "dec": dec[sl],
                "U_a": U,
                "W_a": W,
                "v_t": v,
                "ffn": F,
            }
        )
    res = run_bass_kernel_spmd(nc, in_maps, core_ids=list(range(NCORES)),
                               trace=_trace)

    output = np.empty((B, 1, D), dtype=np.float32)
    context = np.empty((B, 1, E), dtype=np.float32)
    for c in range(NCORES):
        sl = slice(c * NB, (c + 1) * NB)
        output[sl, 0, :] = res.results[c]["out"]
        context[sl, 0, :] = res.results[c]["ctx_out"]
    if _trace:
        return (output, context), res
    return (output, context)


if __name__ == "__main__":
    import reference

    inputs = {k: np.asarray(v) for k, v in reference.setup_inputs().items()}
    (o, c) = kernel(**inputs)
    print("output", o.shape, o.dtype, "context", c.shape, c.dtype)
